# revision 55
# baseline (speedup 1.0000x reference)
"""Chamfer distance loss kernel for 8 Trainium2 NeuronCores.

Problem: template/source point clouds [B=4, N=8192, 3] fp32.
  d2[b,n,m] = ||t[b,n] - s[b,m]||^2
  out = mean_b( (mean_n sqrt(min_m d2) + mean_m sqrt(min_n d2)) / 2 )

Sharding: core c handles batch b=c//2, template-row half h=c%2.  Each
core computes its 4096x8192 slab of the distance matrix once and
extracts BOTH directions from it:
  - row minima (template->source): free-axis min per template row
  - column minima partials (source->template): running elementwise min
    across strips, partition-reduced at the end via PE transpose;
    the two cores sharing a batch are combined on the host.

Per-strip pipeline (strip = 128 template rows):
  PE  : 16 matmuls [128,512] fill PSUM (two 4-bank groups, double-
        buffered), issued as 4 rounds of 4 concurrent K=24 matmuls
        packed into the PE's four 32-row groups (operands replicated at
        base partitions 0/32/64/96).  The contraction emulates fp32 with
        a triple-bf16 split of -2t and s, plus rows carrying |s|^2 and
        |t|^2, so PSUM holds the complete d2 and no bias pass is needed.
  ACT : 4 activations (Copy) drain PSUM -> bf16 strip.
  DVE : a binary tensor_tensor min tree (4096, 2048, 1024) plus a
        narrow accumulating tensor_scalar tail computes the strip's raw
        row-min into out_row[:, s], and one tensor_tensor min folds the
        strip into the running column min.  The narrow tree levels are
        software-pipelined one strip late and every DVE op is issued
        >=2 ops after its producer, so the engine never stalls on its
        own pipe drain.  The last strip's fold widens to fp32 so the
        epilogue needs no copy.  Two fixed ping-pong strip buffers let
        ACT drain strip s+1 while the DVE reduces strip s (a rotating
        tile pool costs ~2.7us/strip in alloc/release overhead).
        Measured end state ~220-270us/iter: at the PE streaming floor
        (262144 moving-operand columns/core at the fixed 1.2 GHz PE
        clock of this platform = ~218us).

Measured per-op rates that drove this structure (this silicon):
  MM bf16/f32r [128,512]           ~0.5 us  (PE effectively 1.2 GHz)
  ACT          [128,2048] psum     ~1.8 us
  TT min bf16  [128,N] sbuf        ~N/2 cyc (2x mode)
  TS/TT with a wide accum_out      1x mode  - never reduce wide with an
        accumulator; tree-reduce with TT first (the v1 kernel's in-place
        8192-wide accumulating tensor_scalar was the main bottleneck)
  DVE TS drain of PSUM fp32        ~2.1 us  (1x; ACT drains instead,
        freeing the DVE for the min work it alone can do)

Column epilogue: fp32 colp is PE-transposed in 64 [128,128] blocks into
PSUM and min-reduced to [128,64]; host combines core pairs.
"""

import numpy as np

B = 4
N = 8192  # points per cloud
HALF = N // 2  # template rows per core
N_CORES = 8
STRIPS = HALF // 128  # 32
M_TILES = N // 512  # 16
K_ROWS = 24  # bf16 triple-split contraction (incl. b2 and a2 rows)
CBLK = N // 128  # 64 column-min output blocks

_cache = {}


def _build_bass(reps=1, ablate=()):
    """ablate: subset of {'colp','drain','mm','epi'} to drop pieces
    for timing ablation (results are garbage when non-empty)."""
    import contextlib
    from concourse import bacc, mybir, tile, masks

    f32 = mybir.dt.float32
    bf16 = mybir.dt.bfloat16
    AOp = mybir.AluOpType
    AFT = mybir.ActivationFunctionType

    ablate = set(ablate)
    if "drain" in ablate:
        ablate |= {"colp", "rowred"}
    if "colp" in ablate:
        ablate |= {"epi"}

    nc = bacc.Bacc("TRN2", target_bir_lowering=False, debug=False,
                   num_devices=N_CORES)

    lhs = nc.dram_tensor("lhs", [128, HALF], bf16,
                         kind="ExternalInput").ap()
    rhs = nc.dram_tensor("rhs", [128, N], bf16,
                         kind="ExternalInput").ap()
    out_row = nc.dram_tensor("out_row", [128, STRIPS], f32,
                             kind="ExternalOutput").ap()
    out_col = nc.dram_tensor("out_col", [128, CBLK], f32,
                             kind="ExternalOutput").ap()

    with tile.TileContext(nc) as tc:
        with tc.tile_pool(name="const", bufs=1) as cpool, \
             tc.tile_pool(name="psum", bufs=1, space="PSUM") as ppool:

            lhs_sb = cpool.tile([128, HALF], bf16, tag="lhs")
            rhs_sb = cpool.tile([128, N], bf16, tag="rhs")
            strips = [cpool.tile([128, N], bf16, name=f"strip{i}",
                                 tag=f"strip{i}") for i in range(2)]
            rscr = cpool.tile([128, N], bf16, tag="rscr")
            gjunk = cpool.tile([128, N], bf16, tag="gjunk")
            ident = cpool.tile([128, 128], f32, tag="ident")
            colp = cpool.tile([128, N], bf16, tag="colp")
            colpf = cpool.tile([128, N], f32, tag="colpf")
            rowacc = cpool.tile([128, STRIPS], f32, tag="rowacc")
            ocol_sb = cpool.tile([128, CBLK], f32, tag="ocol")

            nc.sync.dma_start(lhs_sb[:, :], lhs)
            nc.sync.dma_start(rhs_sb[:, :], rhs)
            masks.make_identity(nc, ident[:, :])
            if ablate:
                # keep every output/read defined under any ablation combo
                nc.vector.memset(rowacc[:, :], 0.0)
                nc.vector.memset(ocol_sb[:, :], 0.0)
                nc.vector.memset(colpf[:, :], 0.0)
                nc.vector.memset(colp[:, :], 0.0)

            # One PSUM tensor spanning all 8 banks, viewed [128, 32, 128]:
            # matmul tiles are 4 slots, drains cover 16 slots, and the
            # epilogue reuses slots 16..31 as transposed blocks.
            P = ppool.tile([128, 32, 128], f32, tag="P")
            if "mm" in ablate and "drain" not in ablate:
                nc.vector.memset(P[:, :, :], 0.0)

            def row_l2(s):
                off = 4096 * (s % 2)
                nc.vector.tensor_tensor(
                    gjunk[:, off + 0: off + 2048],
                    rscr[:, off: off + 2048],
                    rscr[:, off + 2048: off + 4096],
                    AOp.min)

            def row_l3(s):
                off = 4096 * (s % 2)
                nc.vector.tensor_tensor(
                    gjunk[:, off + 2048: off + 3072],
                    gjunk[:, off: off + 1024],
                    gjunk[:, off + 1024: off + 2048],
                    AOp.min)

            def row_tail(s):
                # 1024-wide accumulating tail (wide accumulators fall to
                # 1x mode, so keep this narrow)
                off = 4096 * (s % 2)
                nc.vector.tensor_scalar(
                    gjunk[:, off + 3072: off + 4096],
                    gjunk[:, off + 2048: off + 3072],
                    3.0e38, None, AOp.min, AOp.min,
                    accum_out=rowacc[:, s:s + 1],
                )

            loop_ctx = (tc.For_i(0, reps, 1) if reps > 1
                        else contextlib.nullcontext())
            with loop_ctx:
                # Two fixed ping-pong strip buffers (a rotating tile pool
                # costs ~2.7us/strip in alloc/release overhead): ACT drains
                # strip s+1 while the DVE row-mins and folds strip s.
                for s in range(STRIPS):
                    strip_sb = strips[s % 2]
                    for g in range(4):
                        base = 16 * (g % 2)  # PSUM slot of this 4-bank group
                        if "mm" not in ablate:
                            # 4 concurrent K=24 matmuls packed into the four
                            # 32-row groups of the PE array (operands are
                            # replicated at base partitions 0/32/64/96).
                            for j in range(4):
                                m = 4 * g + j
                                nc.tensor.matmul(
                                    P[:, base + 4 * j: base + 4 * (j + 1), :],
                                    lhsT=lhs_sb[32 * j: 32 * j + K_ROWS,
                                                128 * s: 128 * (s + 1)],
                                    rhs=rhs_sb[32 * j: 32 * j + K_ROWS,
                                               512 * m: 512 * (m + 1)],
                                    start=True, stop=True,
                                    tile_position=(32 * j, 0),
                                )
                        if "drain" not in ablate:
                            # ACT drains PSUM -> bf16 strip (d2 is complete
                            # in PSUM: a2/b2 ride extra contraction rows)
                            nc.scalar.activation(
                                strip_sb[:, 2048 * g: 2048 * (g + 1)],
                                P[:, base: base + 16, :],
                                AFT.Copy,
                            )
                    # row-min: binary TT tree (2x mode; wide accumulators
                    # fall to 1x).  The narrow levels run one strip late
                    # and every level is issued >=2 DVE ops after its
                    # producer, so the engine never stalls on its own
                    # pipe drain.  rscr/gjunk halves ping-pong by parity.
                    off = 4096 * (s % 2)
                    if "rowred" not in ablate:
                        nc.vector.tensor_tensor(
                            rscr[:, off:off + 4096],
                            strip_sb[:, 0:4096], strip_sb[:, 4096:8192],
                            AOp.min)
                        if s > 0:
                            row_l3(s - 1)
                    if "colp" not in ablate:
                        if s == 0:
                            pass  # strip 0 is folded together with strip 1
                        elif s == 1:
                            nc.vector.tensor_tensor(
                                colp[:, :], strips[0][:, :], strip_sb[:, :],
                                AOp.min,
                            )
                        elif s == STRIPS - 1:
                            # final fold widens to fp32 for the transposes
                            nc.vector.tensor_tensor(
                                colpf[:, :], colp[:, :], strip_sb[:, :],
                                AOp.min,
                            )
                        else:
                            nc.vector.tensor_tensor(
                                colp[:, :], colp[:, :], strip_sb[:, :],
                                AOp.min,
                            )
                    if "rowred" not in ablate:
                        if s > 0:
                            row_tail(s - 1)
                        row_l2(s)

                if "rowred" not in ablate:
                    # flush the one-strip-late tail of the pipeline
                    row_l3(STRIPS - 1)
                    row_tail(STRIPS - 1)

                if "epi" not in ablate:
                    # column epilogue: partition-min via PE transpose (fp32)
                    # (only PSUM slots 16..31, so the next iteration's
                    # even-group matmuls don't stall behind the epilogue)
                    for r in range(4):
                        for t in range(16):
                            blk = 16 * r + t
                            nc.tensor.transpose(
                                P[:, 16 + t, :],
                                colpf[:, 128 * blk: 128 * (blk + 1)],
                                ident[:, :],
                            )
                        nc.vector.tensor_reduce(
                            ocol_sb[:, 16 * r: 16 * (r + 1)], P[:, 16:32, :],
                            axis=mybir.AxisListType.X, op=AOp.min,
                        )

                nc.sync.dma_start(out_row, rowacc[:, :])
                nc.sync.dma_start(out_col, ocol_sb[:, :])

    nc.compile()
    return nc


def _split3(x):
    """Exact-ish triple bf16 split: x ~= h + m + l with ~24 mantissa bits."""
    import ml_dtypes
    bf = ml_dtypes.bfloat16
    x = np.ascontiguousarray(x, np.float32)
    h = x.astype(bf)
    r = (x - h.astype(np.float32)).astype(np.float32)
    m = r.astype(bf)
    l = (r - m.astype(np.float32)).astype(bf)
    return h, m, l


def _sq(x):  # |x|^2 per point, fp32
    return (x * x).sum(axis=-1, dtype=np.float32)


def _prep_core_inputs(template, source, c):
    b, h = divmod(c, 2)
    tch = template[b, h * HALF:(h + 1) * HALF]  # [4096, 3] rows
    sfull = source[b]  # [8192, 3] cols

    # Triple bf16 split emulating fp32: v.w ~= vh(wh+wm+wl) + vm(wh+wm)
    # + vl.wh, dropping O(2^-27) cross terms.  b2 (|s|^2) rides ones rows
    # on the stationary side; a2 (|t|^2) rides ones rows on the moving
    # side, so the PE emits the complete d2 with no bias pass.
    import ml_dtypes
    bf = ml_dtypes.bfloat16
    v = (-2.0 * tch.T).astype(np.float32)  # [3, n]
    ones_l = np.ones((1, HALF), bf)
    vh, vm, vl = _split3(v)
    a2h, a2m, a2l = _split3(_sq(tch)[None])  # [1, n]
    lhs = np.ascontiguousarray(np.concatenate(
        [vh, vh, vh, vm, vm, vl,
         ones_l, ones_l, ones_l, a2h, a2m, a2l], axis=0))

    w = np.ascontiguousarray(sfull.T, np.float32)  # [3, m]
    b2 = _sq(sfull)[None]  # [1, m]
    wh, wm, wl = _split3(w)
    b2h, b2m, b2l = _split3(b2)
    ones_r = np.ones((1, N), bf)
    rhs = np.ascontiguousarray(np.concatenate(
        [wh, wm, wl, wh, wm, wh,
         b2h, b2m, b2l, ones_r, ones_r, ones_r], axis=0))

    # Replicate the 24 contraction rows at base partitions 0/32/64/96 so
    # four matmuls can run concurrently in the PE's four 32-row groups.
    lhs4 = np.zeros((128, HALF), bf)
    rhs4 = np.zeros((128, N), bf)
    for j in range(4):
        lhs4[32 * j: 32 * j + K_ROWS] = lhs
        rhs4[32 * j: 32 * j + K_ROWS] = rhs

    return {"lhs": lhs4, "rhs": rhs4}


def _run(template, source, trace=False):
    from concourse.bass_utils import run_bass_kernel_spmd

    template = np.asarray(template, np.float32)
    source = np.asarray(source, np.float32)
    assert template.shape == (B, N, 3) and source.shape == (B, N, 3)

    if "nc" not in _cache:
        _cache["nc"] = _build_bass()
    nc = _cache["nc"]

    in_maps = [_prep_core_inputs(template, source, c) for c in range(N_CORES)]
    res = run_bass_kernel_spmd(nc, in_maps, core_ids=list(range(N_CORES)),
                               trace=trace)

    rows = np.stack([np.asarray(r["out_row"], np.float64)
                     for r in res.results])  # [8, 128, 32] raw d2 rowmins
    cols = np.stack([np.asarray(r["out_col"], np.float64)
                     for r in res.results])  # [8, 128, 64]
    per_core_row = np.sqrt(np.maximum(rows, 0.0)).sum(axis=(1, 2))  # [8]
    cost01 = per_core_row.reshape(B, 2).sum(axis=1) / N  # [B]
    colmin = np.minimum(cols[0::2], cols[1::2])  # [B, 128, 64] raw d2
    cost10 = np.sqrt(np.maximum(colmin, 0.0)).mean(axis=(1, 2))
    chamfer = ((cost01 + cost10) / 2.0).mean()
    return np.asarray(chamfer, dtype=np.float32), res


def kernel(template, source):
    val, _ = _run(template, source, trace=False)
    return val


# revision 56
# speedup vs baseline: 1.2719x; 1.2719x over previous
"""Chamfer distance loss kernel for 8 Trainium2 NeuronCores.

Problem: template/source point clouds [B=4, N=8192, 3] fp32.
  d2[b,n,m] = ||t[b,n] - s[b,m]||^2
  out = mean_b( (mean_n sqrt(min_m d2) + mean_m sqrt(min_n d2)) / 2 )

Sharding: core c handles batch b=c//2, template-row half h=c%2.  Each
core computes its 4096x8192 slab of the distance matrix once and
extracts BOTH directions from it:
  - row minima (template->source): free-axis min per template row
  - column minima partials (source->template): running elementwise min
    across strips, partition-reduced at the end via PE transpose;
    the two cores sharing a batch are combined on the host.

Per-strip pipeline (strip = 128 template rows):
  PE  : 16 matmuls [128,512] fill PSUM (two 4-bank groups, double-
        buffered), issued as 4 rounds of 4 concurrent K=24 matmuls
        packed into the PE's four 32-row groups (operands replicated at
        base partitions 0/32/64/96).  The contraction emulates fp32 with
        a triple-bf16 split of -2t and s, plus rows carrying |s|^2 and
        |t|^2, so PSUM holds the complete d2 and no bias pass is needed.
  ACT : 4 activations (Copy) drain PSUM -> bf16 strip.
  DVE : a binary tensor_tensor min tree (4096, 2048, 1024) plus a
        narrow accumulating tensor_scalar tail computes the strip's raw
        row-min into out_row[:, s], and one tensor_tensor min folds the
        strip into the running column min.  The narrow tree levels are
        software-pipelined one strip late and every DVE op is issued
        >=2 ops after its producer, so the engine never stalls on its
        own pipe drain.  The last strip's fold widens to fp32 so the
        epilogue needs no copy.  Two fixed ping-pong strip buffers let
        ACT drain strip s+1 while the DVE reduces strip s (a rotating
        tile pool costs ~2.7us/strip in alloc/release overhead).
        Measured end state ~220-270us/iter: at the PE streaming floor
        (262144 moving-operand columns/core at the fixed 1.2 GHz PE
        clock of this platform = ~218us).

Measured per-op rates that drove this structure (this silicon):
  MM bf16/f32r [128,512]           ~0.5 us  (PE effectively 1.2 GHz)
  ACT          [128,2048] psum     ~1.8 us
  TT min bf16  [128,N] sbuf        ~N/2 cyc (2x mode)
  TS/TT with a wide accum_out      1x mode  - never reduce wide with an
        accumulator; tree-reduce with TT first (the v1 kernel's in-place
        8192-wide accumulating tensor_scalar was the main bottleneck)
  DVE TS drain of PSUM fp32        ~2.1 us  (1x; ACT drains instead,
        freeing the DVE for the min work it alone can do)

Column epilogue: fp32 colp is PE-transposed in 64 [128,128] blocks into
PSUM and min-reduced to [128,64]; host combines core pairs.
"""

import numpy as np

B = 4
N = 8192  # points per cloud
HALF = N // 2  # template rows per core
N_CORES = 8
STRIPS = HALF // 128  # 32
M_TILES = N // 512  # 16
K_ROWS = 24  # bf16 triple-split contraction (incl. b2 and a2 rows)
CBLK = N // 128  # 64 column-min output blocks

_cache = {}


def _build_bass(reps=1, ablate=()):
    """ablate: subset of {'colp','drain','mm','epi'} to drop pieces
    for timing ablation (results are garbage when non-empty)."""
    import contextlib
    from concourse import bacc, mybir, tile, masks

    f32 = mybir.dt.float32
    bf16 = mybir.dt.bfloat16
    AOp = mybir.AluOpType
    AFT = mybir.ActivationFunctionType

    ablate = set(ablate)
    if "drain" in ablate:
        ablate |= {"colp", "rowred"}
    if "colp" in ablate:
        ablate |= {"epi"}

    nc = bacc.Bacc("TRN2", target_bir_lowering=False, debug=False,
                   num_devices=N_CORES)

    lhs = nc.dram_tensor("lhs", [128, HALF], bf16,
                         kind="ExternalInput").ap()
    rhs = nc.dram_tensor("rhs", [128, N], bf16,
                         kind="ExternalInput").ap()
    out_row = nc.dram_tensor("out_row", [128, STRIPS], f32,
                             kind="ExternalOutput").ap()
    out_col = nc.dram_tensor("out_col", [128, CBLK], f32,
                             kind="ExternalOutput").ap()

    with tile.TileContext(nc) as tc:
        with tc.tile_pool(name="const", bufs=1) as cpool, \
             tc.tile_pool(name="psum", bufs=1, space="PSUM") as ppool:

            lhs_sb = cpool.tile([128, HALF], bf16, tag="lhs")
            rhs_sb = cpool.tile([128, N], bf16, tag="rhs")
            strips = [cpool.tile([128, N], bf16, name=f"strip{i}",
                                 tag=f"strip{i}") for i in range(2)]
            rscr = cpool.tile([128, N], bf16, tag="rscr")
            gjunk = cpool.tile([128, N], bf16, tag="gjunk")
            ident = cpool.tile([128, 128], f32, tag="ident")
            colp = cpool.tile([128, N], bf16, tag="colp")
            colpf = cpool.tile([128, N], f32, tag="colpf")
            rowacc = cpool.tile([128, STRIPS], f32, tag="rowacc")
            ocol_sb = cpool.tile([128, CBLK], f32, tag="ocol")

            nc.sync.dma_start(lhs_sb[:, :], lhs)
            nc.sync.dma_start(rhs_sb[:, :], rhs)
            masks.make_identity(nc, ident[:, :])
            if ablate:
                # keep every output/read defined under any ablation combo
                nc.vector.memset(rowacc[:, :], 0.0)
                nc.vector.memset(ocol_sb[:, :], 0.0)
                nc.vector.memset(colpf[:, :], 0.0)
                nc.vector.memset(colp[:, :], 0.0)

            # One PSUM tensor spanning all 8 banks, viewed [128, 32, 128]:
            # matmul tiles are 4 slots, drains cover 16 slots, and the
            # epilogue reuses slots 16..31 as transposed blocks.
            P = ppool.tile([128, 32, 128], f32, tag="P")
            if "mm" in ablate and "drain" not in ablate:
                nc.vector.memset(P[:, :, :], 0.0)

            def row_l2(s):
                off = 4096 * (s % 2)
                nc.vector.tensor_tensor(
                    gjunk[:, off + 0: off + 2048],
                    rscr[:, off: off + 2048],
                    rscr[:, off + 2048: off + 4096],
                    AOp.min)

            def row_l3(s):
                off = 4096 * (s % 2)
                nc.vector.tensor_tensor(
                    gjunk[:, off + 2048: off + 3072],
                    gjunk[:, off: off + 1024],
                    gjunk[:, off + 1024: off + 2048],
                    AOp.min)

            def row_tail(s):
                # 1024-wide accumulating tail (wide accumulators fall to
                # 1x mode, so keep this narrow)
                off = 4096 * (s % 2)
                nc.vector.tensor_scalar(
                    gjunk[:, off + 3072: off + 4096],
                    gjunk[:, off + 2048: off + 3072],
                    3.0e38, None, AOp.min, AOp.min,
                    accum_out=rowacc[:, s:s + 1],
                )

            loop_ctx = (tc.For_i(0, reps, 1) if reps > 1
                        else contextlib.nullcontext())
            with loop_ctx:
                # Two fixed ping-pong strip buffers (a rotating tile pool
                # costs ~2.7us/strip in alloc/release overhead): ACT drains
                # strip s+1 while the DVE row-mins and folds strip s.
                for s in range(STRIPS):
                    strip_sb = strips[s % 2]
                    for g in range(4):
                        base = 16 * (g % 2)  # PSUM slot of this 4-bank group
                        if "mm" not in ablate:
                            # 4 concurrent K=24 matmuls packed into the four
                            # 32-row groups of the PE array (operands are
                            # replicated at base partitions 0/32/64/96).
                            for j in range(4):
                                m = 4 * g + j
                                nc.tensor.matmul(
                                    P[:, base + 4 * j: base + 4 * (j + 1), :],
                                    lhsT=lhs_sb[32 * j: 32 * j + K_ROWS,
                                                128 * s: 128 * (s + 1)],
                                    rhs=rhs_sb[32 * j: 32 * j + K_ROWS,
                                               512 * m: 512 * (m + 1)],
                                    start=True, stop=True,
                                    tile_position=(32 * j, 0),
                                )
                        if "drain" not in ablate:
                            # ACT drains PSUM -> bf16 strip (d2 is complete
                            # in PSUM: a2/b2 ride extra contraction rows)
                            nc.scalar.activation(
                                strip_sb[:, 2048 * g: 2048 * (g + 1)],
                                P[:, base: base + 16, :],
                                AFT.Copy,
                            )
                    # row-min: binary TT tree (2x mode; wide accumulators
                    # fall to 1x).  The narrow levels run one strip late
                    # and every level is issued >=2 DVE ops after its
                    # producer, so the engine never stalls on its own
                    # pipe drain.  rscr/gjunk halves ping-pong by parity.
                    off = 4096 * (s % 2)
                    if "rowred" not in ablate:
                        nc.vector.tensor_tensor(
                            rscr[:, off:off + 4096],
                            strip_sb[:, 0:4096], strip_sb[:, 4096:8192],
                            AOp.min)
                        if s > 0:
                            row_l3(s - 1)
                    if "colp" not in ablate:
                        if s == 0:
                            pass  # strip 0 is folded together with strip 1
                        elif s == 1:
                            nc.vector.tensor_tensor(
                                colp[:, :], strips[0][:, :], strip_sb[:, :],
                                AOp.min,
                            )
                        elif s == STRIPS - 1:
                            # final fold widens to fp32 for the transposes,
                            # chunked so each epilogue transpose round can
                            # start as soon as its quarter of colpf is ready
                            for c in range(4):
                                nc.vector.tensor_tensor(
                                    colpf[:, 2048 * c: 2048 * (c + 1)],
                                    colp[:, 2048 * c: 2048 * (c + 1)],
                                    strip_sb[:, 2048 * c: 2048 * (c + 1)],
                                    AOp.min,
                                )
                        else:
                            nc.vector.tensor_tensor(
                                colp[:, :], colp[:, :], strip_sb[:, :],
                                AOp.min,
                            )
                    if "rowred" not in ablate:
                        if s > 0:
                            row_tail(s - 1)
                        row_l2(s)

                if "rowred" not in ablate:
                    # flush the one-strip-late tail of the pipeline
                    row_l3(STRIPS - 1)
                    row_tail(STRIPS - 1)

                if "epi" not in ablate:
                    # column epilogue: partition-min via PE transpose (fp32)
                    # (only PSUM slots 16..31, so the next iteration's
                    # even-group matmuls don't stall behind the epilogue)
                    for r in range(4):
                        for t in range(16):
                            blk = 16 * r + t
                            nc.tensor.transpose(
                                P[:, 16 + t, :],
                                colpf[:, 128 * blk: 128 * (blk + 1)],
                                ident[:, :],
                            )
                        nc.vector.tensor_reduce(
                            ocol_sb[:, 16 * r: 16 * (r + 1)], P[:, 16:32, :],
                            axis=mybir.AxisListType.X, op=AOp.min,
                        )

                nc.sync.dma_start(out_row, rowacc[:, :])
                nc.sync.dma_start(out_col, ocol_sb[:, :])

    nc.compile()
    return nc


def _split3(x):
    """Exact-ish triple bf16 split: x ~= h + m + l with ~24 mantissa bits."""
    import ml_dtypes
    bf = ml_dtypes.bfloat16
    x = np.ascontiguousarray(x, np.float32)
    h = x.astype(bf)
    r = (x - h.astype(np.float32)).astype(np.float32)
    m = r.astype(bf)
    l = (r - m.astype(np.float32)).astype(bf)
    return h, m, l


def _sq(x):  # |x|^2 per point, fp32
    return (x * x).sum(axis=-1, dtype=np.float32)


def _prep_core_inputs(template, source, c):
    b, h = divmod(c, 2)
    tch = template[b, h * HALF:(h + 1) * HALF]  # [4096, 3] rows
    sfull = source[b]  # [8192, 3] cols

    # Triple bf16 split emulating fp32: v.w ~= vh(wh+wm+wl) + vm(wh+wm)
    # + vl.wh, dropping O(2^-27) cross terms.  b2 (|s|^2) rides ones rows
    # on the stationary side; a2 (|t|^2) rides ones rows on the moving
    # side, so the PE emits the complete d2 with no bias pass.
    import ml_dtypes
    bf = ml_dtypes.bfloat16
    v = (-2.0 * tch.T).astype(np.float32)  # [3, n]
    ones_l = np.ones((1, HALF), bf)
    vh, vm, vl = _split3(v)
    a2h, a2m, a2l = _split3(_sq(tch)[None])  # [1, n]
    lhs = np.ascontiguousarray(np.concatenate(
        [vh, vh, vh, vm, vm, vl,
         ones_l, ones_l, ones_l, a2h, a2m, a2l], axis=0))

    w = np.ascontiguousarray(sfull.T, np.float32)  # [3, m]
    b2 = _sq(sfull)[None]  # [1, m]
    wh, wm, wl = _split3(w)
    b2h, b2m, b2l = _split3(b2)
    ones_r = np.ones((1, N), bf)
    rhs = np.ascontiguousarray(np.concatenate(
        [wh, wm, wl, wh, wm, wh,
         b2h, b2m, b2l, ones_r, ones_r, ones_r], axis=0))

    # Replicate the 24 contraction rows at base partitions 0/32/64/96 so
    # four matmuls can run concurrently in the PE's four 32-row groups.
    lhs4 = np.zeros((128, HALF), bf)
    rhs4 = np.zeros((128, N), bf)
    for j in range(4):
        lhs4[32 * j: 32 * j + K_ROWS] = lhs
        rhs4[32 * j: 32 * j + K_ROWS] = rhs

    return {"lhs": lhs4, "rhs": rhs4}


def _run(template, source, trace=False):
    from concourse.bass_utils import run_bass_kernel_spmd

    template = np.asarray(template, np.float32)
    source = np.asarray(source, np.float32)
    assert template.shape == (B, N, 3) and source.shape == (B, N, 3)

    if "nc" not in _cache:
        _cache["nc"] = _build_bass()
    nc = _cache["nc"]

    in_maps = [_prep_core_inputs(template, source, c) for c in range(N_CORES)]
    res = run_bass_kernel_spmd(nc, in_maps, core_ids=list(range(N_CORES)),
                               trace=trace)

    rows = np.stack([np.asarray(r["out_row"], np.float64)
                     for r in res.results])  # [8, 128, 32] raw d2 rowmins
    cols = np.stack([np.asarray(r["out_col"], np.float64)
                     for r in res.results])  # [8, 128, 64]
    per_core_row = np.sqrt(np.maximum(rows, 0.0)).sum(axis=(1, 2))  # [8]
    cost01 = per_core_row.reshape(B, 2).sum(axis=1) / N  # [B]
    colmin = np.minimum(cols[0::2], cols[1::2])  # [B, 128, 64] raw d2
    cost10 = np.sqrt(np.maximum(colmin, 0.0)).mean(axis=(1, 2))
    chamfer = ((cost01 + cost10) / 2.0).mean()
    return np.asarray(chamfer, dtype=np.float32), res


def kernel(template, source):
    val, _ = _run(template, source, trace=False)
    return val
